# revision 8
# baseline (speedup 1.0000x reference)
"""Causal self-attention (B=2, T=2048, C=768, H=12) on 8 NeuronCores.

Sharding: batch x head tensor-parallel, zero device collectives. Core
d = 4b + g handles batch b and heads {3g, 3g+1, 3g+2}. Each core:
  - computes Q^T/K^T/V for ONLY its 3 heads (x@W_qkv column slice),
  - runs full causal attention for those heads over all T=2048 queries,
  - computes the row-parallel output-projection partial
    z_d = Y_d @ W_proj[rows of its heads]  ->  [T, C] fp16.
Host unshard: out[b] = sum_g z_{4b+g}  (the standard row-parallel
linear reduce, done on host in fp32).

On-device pipeline (bf16 matmul operands, fp32 PSUM accumulation):
  x -> PE-transpose -> xT;  QKV^T tiles T1=[q0|q1], T2=[k0|k1],
  T3=[q2|k2] (pairs of 64-dim heads per 128 partitions), V[t] [128,3,65]
  with a ones column folding the softmax denominator into PV row 64.
  S^T[k,q] per (head, 256-query chunk, 128-key tile); exp on ACT
  (scale=1/8, no max-sub: |logits| small so fp32 exp is safe); causal
  mask multiply (host-built [128,2,256] pair mask) only on the two
  diagonal key tiles of each chunk; PV accumulates [65, 2, 256] per
  chunk pair; reciprocal + ones-matmul broadcast + multiply normalizes
  into Y^T; projection contract over [YT01 (128) | YT2 (64)] with the
  fp16 cast split ACT/DVE, DMA out per 128-row tile.

Engine budget (cost model): PE ~87us, ACT ~67us, DVE ~50us.
"""

import numpy as np
import ml_dtypes

B, T, C, H, D = 2, 2048, 768, 12, 64
NCORES = 8
HPC = 3            # heads per core
CT = C // 128      # 6 contraction tiles
TT = T // 128      # 16 row tiles
QW = 256           # query chunk width
NCH = T // QW      # 8 query chunks

_CACHE = {}


def _build_program(with_bias=True):
    import os
    import concourse.bass as bass
    import concourse.bacc as bacc
    import concourse.mybir as mybir
    import concourse.tile as tile

    F32 = mybir.dt.float32
    F32R = mybir.dt.float32r
    BF16 = mybir.dt.bfloat16
    FP16 = mybir.dt.float16
    AF = mybir.ActivationFunctionType

    nc = bacc.Bacc()
    xbf = nc.declare_dram_parameter("xbf", [T, C], BF16, isOutput=False)
    wq = nc.declare_dram_parameter("wq", [C, 576], BF16, isOutput=False)
    wpd = nc.declare_dram_parameter("wp", [192, C], BF16, isOutput=False)
    maskp = nc.declare_dram_parameter("maskp", [128, 2, QW], BF16,
                                      isOutput=False)
    ident_in = nc.declare_dram_parameter("ident_in", [128, 128], BF16,
                                         isOutput=False)
    if with_bias:
        bqd = nc.declare_dram_parameter("bq", [1, 576], BF16, isOutput=False)
        bpd = nc.declare_dram_parameter("bp", [1, C], BF16, isOutput=False)
    z_out = nc.declare_dram_parameter("z", [T, C], FP16, isOutput=True)

    scale = 1.0 / float(np.sqrt(D))

    with tile.TileContext(nc) as tc:
        with tc.tile_pool(name="const", bufs=1) as constp, \
             tc.tile_pool(name="data", bufs=1) as datap, \
             tc.tile_pool(name="pt", bufs=4) as ptp, \
             tc.tile_pool(name="small", bufs=3) as smallp, \
             tc.tile_pool(name="zs", bufs=2) as zsp, \
             tc.tile_pool(name="psa", bufs=2, space="PSUM") as psa, \
             tc.tile_pool(name="pss", bufs=3, space="PSUM") as pss, \
             tc.tile_pool(name="pso", bufs=3, space="PSUM") as pso:

            # ---- constants ------------------------------------------------
            idn = constp.tile([128, 128], BF16, tag="idn")
            wq_s = constp.tile([128, CT, 576], BF16, tag="wq")
            wp01 = constp.tile([128, C], BF16, tag="wp01")
            wp2 = constp.tile([64, C], BF16, tag="wp2")
            mask_s = constp.tile([128, 2, QW], BF16, tag="mask")
            ones1 = constp.tile([1, 64], BF16, tag="ones1")
            nc.vector.memset(ones1, 1.0)
            if with_bias:
                bq_s = constp.tile([1, 576], BF16, tag="bq")
                bp_s = constp.tile([1, C], BF16, tag="bp")
                onesr = constp.tile([1, 512], BF16, tag="onesr")
                nc.vector.memset(onesr, 1.0)

            # ---- persistent data ------------------------------------------
            xl = datap.tile([128, TT, C], BF16, tag="xl")
            xT = [datap.tile([128, T], BF16, tag=f"xT{c}", name=f"xT{c}")
                  for c in range(CT)]
            T1 = datap.tile([128, T], BF16, tag="T1", name="T1")
            T2 = datap.tile([128, T], BF16, tag="T2", name="T2")
            T3 = datap.tile([128, T], BF16, tag="T3", name="T3")
            V = [datap.tile([128, HPC, D + 1], BF16, tag=f"V{t}",
                            name=f"V{t}")
                 for t in range(TT)]
            YT01 = datap.tile([128, T], BF16, tag="YT01", name="YT01")
            YT2 = datap.tile([64, T], BF16, tag="YT2", name="YT2")
            K2D = datap.tile([64, T], BF16, tag="K2D", name="K2D")

            # ---- phase A: DMAs in priority order --------------------------
            nc.sync.dma_start(out=idn, in_=ident_in[:, :])
            x_ap = xbf[:, :]
            for m in range(4):
                nc.sync.dma_start(
                    out=xl[:, 4 * m:4 * (m + 1), :],
                    in_=bass.AP(tensor=x_ap.tensor,
                                offset=x_ap.offset + 512 * m * C,
                                ap=[[C, 128], [128 * C, 4], [1, C]]),
                )
            w_ap = wq[:, :]
            nc.gpsimd.dma_start(
                out=wq_s,
                in_=bass.AP(tensor=w_ap.tensor, offset=w_ap.offset,
                            ap=[[576, 128], [128 * 576, CT], [1, 576]]),
            )
            nc.gpsimd.dma_start(out=wp01, in_=wpd[0:128, :])
            nc.gpsimd.dma_start(out=wp2, in_=wpd[128:192, :])
            nc.gpsimd.dma_start(out=mask_s, in_=maskp[:, :, :])
            if with_bias:
                nc.gpsimd.dma_start(out=bq_s, in_=bqd[:, :])
                nc.gpsimd.dma_start(out=bp_s, in_=bpd[:, :])

            # ---- phase B: transpose x, QKV projections --------------------
            for m in range(4):
                tsl = slice(512 * m, 512 * (m + 1))
                for c in range(CT):
                    tpb = pss.tile([128, 4, 128], BF16, tag="s", name="tpb")
                    for t4 in range(4):
                        nc.tensor.transpose(
                            out=tpb[:, t4, :],
                            in_=xl[:, 4 * m + t4, 128 * c:128 * (c + 1)],
                            identity=idn)
                    nc.vector.tensor_copy(out=xT[c][:, tsl], in_=tpb)
                for ti, tl in ((0, T1), (1, T2), (2, T3)):
                    acc = psa.tile([128, 512], F32, tag="acc", name="acc")
                    for c in range(CT):
                        nc.tensor.matmul(
                            out=acc,
                            lhsT=wq_s[:, c, 128 * ti:128 * (ti + 1)],
                            rhs=xT[c][:, tsl],
                            start=(c == 0),
                            stop=(c == CT - 1 and not with_bias))
                    if with_bias:
                        nc.tensor.matmul(
                            out=acc, lhsT=bq_s[0:1, 128 * ti:128 * (ti + 1)],
                            rhs=onesr, start=False, stop=True,
                            skip_group_check=True)
                    nc.scalar.activation(out=tl[:, tsl], in_=acc,
                                         func=AF.Copy)
                for t in range(4 * m, 4 * (m + 1)):
                    accv = psa.tile([128, 512], F32, tag="acc", name="accv")
                    for c in range(CT):
                        nc.tensor.matmul(
                            out=accv[:, 0:192],
                            lhsT=xT[c][:, 128 * t:128 * (t + 1)],
                            rhs=wq_s[:, c, 384:576],
                            start=(c == 0),
                            stop=(c == CT - 1 and not with_bias))
                    if with_bias:
                        nc.tensor.matmul(
                            out=accv[:, 0:192], lhsT=onesr[0:1, 0:128],
                            rhs=bq_s[0:1, 384:576], start=False, stop=True,
                            skip_group_check=True)
                    nc.vector.tensor_copy(out=V[t][:, :, 0:D],
                                          in_=accv[:, 0:192])
                    nc.vector.memset(V[t][:, :, D:D + 1], 1.0)

            # k2 lives at partitions 64-127 of T3 but q2 at 0-63; the PE
            # requires fmap/weights at the same partition base, so mirror
            # k2 down to partitions 0-63 with an SBUF->SBUF DMA.
            nc.gpsimd.dma_start(out=K2D[0:64, :], in_=T3[64:128, :])

            # ---- phase C: attention per head ------------------------------
            # head -> (K tile, K part offset), (Q tile, Q part offset)
            hsl = [((T2, 0), (T1, 0)), ((T2, 64), (T1, 64)),
                   ((K2D, 0), (T3, 0))]
            for h in range(HPC):
                (kb, kp), (qb, qp) = hsl[h]
                ot = None
                for n in range(NCH):
                    qsl = slice(QW * n, QW * (n + 1))
                    if n % 2 == 0:
                        ot = pso.tile([65, 2, QW], F32, tag="ot", name="ot")
                    for pr in range(n + 1):
                        sps = pss.tile([128, 2, QW], F32, tag="s", name="sps")
                        for j in range(2):
                            kt = 2 * pr + j
                            nc.tensor.matmul(
                                out=sps[:, j, :],
                                lhsT=kb[kp:kp + 64, 128 * kt:128 * (kt + 1)],
                                rhs=qb[qp:qp + 64, qsl],
                                start=True, stop=True)
                        pt = ptp.tile([128, 2, QW], BF16, tag="pt",
                                      name="pt")
                        nc.scalar.activation(out=pt, in_=sps, func=AF.Exp,
                                             scale=scale)
                        if pr == n:
                            nc.vector.tensor_mul(pt, pt, mask_s)
                        for j in range(2):
                            nc.tensor.matmul(
                                out=ot[:, n % 2, :],
                                lhsT=V[2 * pr + j][:, h, :],
                                rhs=pt[:, j, :],
                                start=(pr == 0 and j == 0),
                                stop=(pr == n and j == 1),
                                skip_group_check=True)
                    if n % 2 == 1:
                        # normalize chunk pair (n-1, n)
                        rec = smallp.tile([1, 2, QW], BF16, tag="rec",
                                          name="rec")
                        with nc.allow_low_precision(
                                reason="bf16 softmax denom reciprocal"):
                            nc.vector.reciprocal(out=rec,
                                                 in_=ot[64:65, :, :])
                        recb = pss.tile([64, 2, QW], F32, tag="s",
                                        name="recb")
                        nc.tensor.matmul(out=recb, lhsT=ones1, rhs=rec,
                                         start=True, stop=True)
                        recb_sb = smallp.tile([64, 2, QW], F32,
                                              tag="recb_sb", name="recb_sb")
                        nc.scalar.activation(out=recb_sb, in_=recb,
                                             func=AF.Copy)
                        ysl = slice(512 * (n // 2), 512 * (n // 2 + 1))
                        if h < 2:
                            ydst = YT01[64 * h:64 * (h + 1), ysl]
                        else:
                            ydst = YT2[0:64, ysl]
                        nc.vector.tensor_mul(ydst, ot[0:64, :, :], recb_sb)

            # ---- phase D: output projection -------------------------------
            for t in range(TT):
                zt = zsp.tile([128, C], FP16, tag="zt", name="zt")
                for off, w in ((0, 512), (512, 256)):
                    acc = psa.tile([128, 512], F32, tag="acc", name="accp")
                    nc.tensor.matmul(
                        out=acc[:, 0:w],
                        lhsT=YT01[:, 128 * t:128 * (t + 1)],
                        rhs=wp01[:, off:off + w],
                        start=True, stop=False)
                    nc.tensor.matmul(
                        out=acc[:, 0:w],
                        lhsT=YT2[0:64, 128 * t:128 * (t + 1)],
                        rhs=wp2[:, off:off + w],
                        start=False, stop=(not with_bias),
                        skip_group_check=True)
                    if with_bias:
                        nc.tensor.matmul(
                            out=acc[:, 0:w], lhsT=onesr[0:1, 0:128],
                            rhs=bp_s[0:1, off:off + w], start=False,
                            stop=True, skip_group_check=True)
                    if w == 512:
                        nc.scalar.activation(out=zt[:, 0:512],
                                             in_=acc[:, 0:512], func=AF.Copy)
                    else:
                        nc.vector.tensor_copy(out=zt[:, 512:768],
                                              in_=acc[:, 0:256])
                nc.sync.dma_start(out=z_out[128 * t:128 * (t + 1), :],
                                  in_=zt)

    nc.finalize()
    return nc


def _prep_inputs(x, W_qkv, b_qkv, W_proj, b_proj):
    bf16 = ml_dtypes.bfloat16
    x = np.ascontiguousarray(np.asarray(x, dtype=np.float32))
    W_qkv = np.asarray(W_qkv, dtype=np.float32)
    b_qkv = np.asarray(b_qkv, dtype=np.float32)
    W_proj = np.asarray(W_proj, dtype=np.float32)
    b_proj = np.asarray(b_proj, dtype=np.float32)

    xb = [np.ascontiguousarray(x[b].astype(bf16)) for b in range(B)]
    ident = np.eye(128, dtype=np.float32).astype(bf16)

    p = np.arange(128)[:, None]
    q = np.arange(QW)[None, :]
    maskp = np.stack([(q >= p), (q >= p + 128)], axis=1)
    maskp = np.ascontiguousarray(maskp.astype(np.float32).astype(bf16))

    in_maps = []
    for d in range(NCORES):
        b, g = d // 4, d % 4
        ha, hb, hc = 3 * g, 3 * g + 1, 3 * g + 2
        # wq column order: [q_a q_b k_a k_b q_c k_c v_a v_b v_c]
        cols = []
        for blk, h in ((0, ha), (0, hb), (1, ha), (1, hb), (0, hc),
                       (1, hc), (2, ha), (2, hb), (2, hc)):
            base = blk * C + 64 * h
            cols.append(np.arange(base, base + 64))
        cols = np.concatenate(cols)
        wq_d = np.ascontiguousarray(W_qkv[:, cols].astype(bf16))
        rows = np.concatenate([np.arange(64 * h, 64 * h + 64)
                               for h in (ha, hb, hc)])
        wp_d = np.ascontiguousarray(W_proj[rows, :].astype(bf16))
        im = {
            "xbf": xb[b],
            "wq": wq_d,
            "wp": wp_d,
            "maskp": maskp,
            "ident_in": ident,
        }
        if np.any(b_qkv) or np.any(b_proj):
            im["bq"] = np.ascontiguousarray(
                b_qkv[cols].reshape(1, 576).astype(bf16))
            im["bp"] = np.ascontiguousarray(
                b_proj.reshape(1, C).astype(bf16))
        in_maps.append(im)
    return in_maps


def kernel(x, W_qkv, b_qkv, W_proj, b_proj):
    import os
    from concourse.bass_utils import run_bass_kernel_spmd

    in_maps = _prep_inputs(x, W_qkv, b_qkv, W_proj, b_proj)
    with_bias = bool(np.any(np.asarray(b_qkv)) or np.any(np.asarray(b_proj)))
    key = f"nc{with_bias}"
    if key not in _CACHE:
        _CACHE[key] = _build_program(with_bias)
    nc = _CACHE[key]
    res = run_bass_kernel_spmd(nc, in_maps, list(range(NCORES)),
                               trace=os.environ.get("KTRACE", "") == "1")
    _CACHE["last_result"] = res

    out = np.empty((B, T, C), dtype=np.float32)
    for b in range(B):
        acc = None
        for g in range(4):
            z = np.asarray(res.results[4 * b + g]["z"]).astype(np.float32)
            acc = z if acc is None else acc + z
        out[b] = acc
    return out


# revision 10
# speedup vs baseline: 1.0429x; 1.0429x over previous
"""Causal self-attention (B=2, T=2048, C=768, H=12) on 8 NeuronCores.

Sharding: batch x head tensor-parallel, zero device collectives. Core
d = 4b + g handles batch b and heads {3g, 3g+1, 3g+2}. Each core:
  - computes Q^T/K^T/V for ONLY its 3 heads (x@W_qkv column slice),
  - runs full causal attention for those heads over all T=2048 queries,
  - computes the row-parallel output-projection partial
    z_d = Y_d @ W_proj[rows of its heads]  ->  [T, C] fp16.
Host unshard: out[b] = sum_g z_{4b+g}  (the standard row-parallel
linear reduce, done on host in fp32).

On-device pipeline (bf16 matmul operands, fp32 PSUM accumulation):
  x -> PE-transpose -> xT;  QKV^T tiles T1=[q0|q1], T2=[k0|k1],
  T3=[q2|k2] (pairs of 64-dim heads per 128 partitions; k2 mirrored to
  partitions 0-63 by SBUF->SBUF DMA since PE wants fmap/weights at the
  same partition base), V[t] [128,3,65] with a ones column folding the
  softmax denominator into PV row 64.

  Attention runs 256-query chunks OUTER, heads inner, fully pipelined
  with the projection: S^T[k,q] per 128-key tile accumulates into a
  4-key-tile PSUM quad, ONE wide exp per quad on ACT (scale=1/8, no
  max-sub; logits are small so fp32 exp is safe), causal mask multiply
  only on the two diagonal key tiles, PV accumulates [65, 2, 256] per
  chunk pair, reciprocal (bf16) + ones-matmul broadcast + multiply
  normalizes into Y^T, and each finished chunk pair immediately feeds
  its 4 projection row-tiles and fp16 z DMA. This keeps PE fed during
  the ACT-heavy attention inner loop and spreads the output DMA.

Engine budget (cost model): PE ~87us busy, ACT ~67us, DVE ~54us.
"""

import numpy as np
import ml_dtypes

B, T, C, H, D = 2, 2048, 768, 12, 64
NCORES = 8
HPC = 3            # heads per core
CT = C // 128      # 6 contraction tiles
TT = T // 128      # 16 row tiles
QW = 256           # query chunk width
NCH = T // QW      # 8 query chunks

_CACHE = {}


def _build_program(with_bias=True):
    import os
    import concourse.bass as bass
    import concourse.bacc as bacc
    import concourse.mybir as mybir
    import concourse.tile as tile

    F32 = mybir.dt.float32
    BF16 = mybir.dt.bfloat16
    FP16 = mybir.dt.float16
    AF = mybir.ActivationFunctionType

    nc = bacc.Bacc()
    xbf = nc.declare_dram_parameter("xbf", [T, C], BF16, isOutput=False)
    wq = nc.declare_dram_parameter("wq", [C, 576], BF16, isOutput=False)
    wpd = nc.declare_dram_parameter("wp", [192, C], BF16, isOutput=False)
    maskp = nc.declare_dram_parameter("maskp", [128, 2, QW], BF16,
                                      isOutput=False)
    ident_in = nc.declare_dram_parameter("ident_in", [128, 128], BF16,
                                         isOutput=False)
    if with_bias:
        bqd = nc.declare_dram_parameter("bq", [1, 576], BF16, isOutput=False)
        bpd = nc.declare_dram_parameter("bp", [1, C], BF16, isOutput=False)
    z_out = nc.declare_dram_parameter("z", [T, C], FP16, isOutput=True)

    scale = 1.0 / float(np.sqrt(D))

    with tile.TileContext(nc) as tc:
        with tc.tile_pool(name="const", bufs=1) as constp, \
             tc.tile_pool(name="data", bufs=1) as datap, \
             tc.tile_pool(name="pt", bufs=4) as ptp, \
             tc.tile_pool(name="small", bufs=3) as smallp, \
             tc.tile_pool(name="zs", bufs=2) as zsp, \
             tc.tile_pool(name="pss", bufs=2, space="PSUM") as pss, \
             tc.tile_pool(name="pso", bufs=3, space="PSUM") as pso, \
             tc.tile_pool(name="psr", bufs=1, space="PSUM") as psr:

            # ---- constants ------------------------------------------------
            idn = constp.tile([128, 128], BF16, tag="idn")
            wq_s = constp.tile([128, CT, 576], BF16, tag="wq")
            wp01 = constp.tile([128, C], BF16, tag="wp01")
            wp2 = constp.tile([64, C], BF16, tag="wp2")
            mask_s = constp.tile([128, 2, QW], BF16, tag="mask")
            ones1 = constp.tile([1, 64], BF16, tag="ones1")
            nc.vector.memset(ones1, 1.0)
            if with_bias:
                bq_s = constp.tile([1, 576], BF16, tag="bq")
                bp_s = constp.tile([1, C], BF16, tag="bp")
                onesr = constp.tile([1, 512], BF16, tag="onesr")
                nc.vector.memset(onesr, 1.0)

            # ---- persistent data ------------------------------------------
            xl = datap.tile([128, TT, C], BF16, tag="xl")
            xT = [datap.tile([128, T], BF16, tag=f"xT{c}", name=f"xT{c}")
                  for c in range(CT)]
            T1 = datap.tile([128, T], BF16, tag="T1", name="T1")
            T2 = datap.tile([128, T], BF16, tag="T2", name="T2")
            T3 = datap.tile([128, T], BF16, tag="T3", name="T3")
            V = [datap.tile([128, HPC, D + 1], BF16, tag=f"V{t}",
                            name=f"V{t}")
                 for t in range(TT)]
            YT01 = datap.tile([128, T], BF16, tag="YT01", name="YT01")
            YT2 = datap.tile([64, T], BF16, tag="YT2", name="YT2")
            K2D = datap.tile([64, T], BF16, tag="K2D", name="K2D")

            # ---- input DMAs in priority order -----------------------------
            nc.sync.dma_start(out=idn, in_=ident_in[:, :])
            x_ap = xbf[:, :]
            for m in range(4):
                nc.sync.dma_start(
                    out=xl[:, 4 * m:4 * (m + 1), :],
                    in_=bass.AP(tensor=x_ap.tensor,
                                offset=x_ap.offset + 512 * m * C,
                                ap=[[C, 128], [128 * C, 4], [1, C]]),
                )
            w_ap = wq[:, :]
            nc.gpsimd.dma_start(
                out=wq_s,
                in_=bass.AP(tensor=w_ap.tensor, offset=w_ap.offset,
                            ap=[[576, 128], [128 * 576, CT], [1, 576]]),
            )
            nc.gpsimd.dma_start(out=wp01, in_=wpd[0:128, :])
            nc.gpsimd.dma_start(out=wp2, in_=wpd[128:192, :])
            nc.gpsimd.dma_start(out=mask_s, in_=maskp[:, :, :])
            if with_bias:
                nc.gpsimd.dma_start(out=bq_s, in_=bqd[:, :])
                nc.gpsimd.dma_start(out=bp_s, in_=bpd[:, :])

            # head -> (K tile, part offset), (Q tile, part offset)
            hsl = [((T2, 0), (T1, 0)), ((T2, 64), (T1, 64)),
                   ((K2D, 0), (T3, 0))]
            ots = [None] * HPC

            def qkv_block(m):
                """Transpose x chunk m, then QKV projections for chunk m."""
                tsl = slice(512 * m, 512 * (m + 1))
                for c in range(CT):
                    tpb = pss.tile([128, 4, 128], BF16, tag="s", name="tpb")
                    for t4 in range(4):
                        nc.tensor.transpose(
                            out=tpb[:, t4, :],
                            in_=xl[:, 4 * m + t4, 128 * c:128 * (c + 1)],
                            identity=idn)
                    nc.vector.tensor_copy(out=xT[c][:, tsl], in_=tpb)
                for ti, tl in ((0, T1), (1, T2), (2, T3)):
                    acc = pss.tile([128, 512], F32, tag="s", name="acc")
                    for c in range(CT):
                        nc.tensor.matmul(
                            out=acc,
                            lhsT=wq_s[:, c, 128 * ti:128 * (ti + 1)],
                            rhs=xT[c][:, tsl],
                            start=(c == 0),
                            stop=(c == CT - 1 and not with_bias))
                    if with_bias:
                        nc.tensor.matmul(
                            out=acc, lhsT=bq_s[0:1, 128 * ti:128 * (ti + 1)],
                            rhs=onesr, start=False, stop=True,
                            skip_group_check=True)
                    nc.scalar.activation(out=tl[:, tsl], in_=acc,
                                         func=AF.Copy)
                for t in range(4 * m, 4 * (m + 1)):
                    accv = pss.tile([128, 512], F32, tag="s", name="accv")
                    for c in range(CT):
                        nc.tensor.matmul(
                            out=accv[:, 0:192],
                            lhsT=xT[c][:, 128 * t:128 * (t + 1)],
                            rhs=wq_s[:, c, 384:576],
                            start=(c == 0),
                            stop=(c == CT - 1 and not with_bias))
                    if with_bias:
                        nc.tensor.matmul(
                            out=accv[:, 0:192], lhsT=onesr[0:1, 0:128],
                            rhs=bq_s[0:1, 384:576], start=False, stop=True,
                            skip_group_check=True)
                    nc.vector.tensor_copy(out=V[t][:, :, 0:D],
                                          in_=accv[:, 0:192])
                    nc.vector.memset(V[t][:, :, D:D + 1], 1.0)
                # mirror k2 (partitions 64-127 of T3) down to partitions
                # 0-63: PE needs fmap/weights at the same partition base.
                nc.gpsimd.dma_start(out=K2D[0:64, tsl],
                                    in_=T3[64:128, tsl])

            def attn_chunk(h, n):
                """S/exp/PV for head h, query chunk n (k tiles 0..2n+1)."""
                (kb, kp), (qb, qp) = hsl[h]
                qsl = slice(QW * n, QW * (n + 1))
                if n % 2 == 0:
                    ots[h] = pso.tile([65, 2, QW], F32, tag="ot", name="ot")
                ot = ots[h]
                npr = n + 1                      # pairs of key tiles
                pr = 0
                while pr < npr:
                    g = 2 if npr - pr >= 2 else 1    # pairs in this group
                    sps = pss.tile([128, 2 * g, QW], F32, tag="s",
                                   name="sps")
                    for jj in range(2 * g):
                        kt = 2 * pr + jj
                        nc.tensor.matmul(
                            out=sps[:, jj, :],
                            lhsT=kb[kp:kp + 64, 128 * kt:128 * (kt + 1)],
                            rhs=qb[qp:qp + 64, qsl],
                            start=True, stop=True)
                    pt = ptp.tile([128, 2 * g, QW], BF16, tag="pt",
                                  name="pt")
                    nc.scalar.activation(out=pt, in_=sps, func=AF.Exp,
                                         scale=scale)
                    if pr + g - 1 == n:          # group holds diagonal pair
                        dsl = slice(2 * (g - 1), 2 * g)
                        nc.vector.tensor_mul(pt[:, dsl, :], pt[:, dsl, :],
                                             mask_s)
                    for jj in range(2 * g):
                        kt = 2 * pr + jj
                        nc.tensor.matmul(
                            out=ot[:, n % 2, :],
                            lhsT=V[kt][:, h, :],
                            rhs=pt[:, jj, :],
                            start=(kt == 0),
                            stop=(kt == 2 * n + 1),
                            skip_group_check=True)
                    pr += g

            def normalize(h, n):
                """Normalize head h for the chunk pair (n-1, n), n odd."""
                ot = ots[h]
                rec = smallp.tile([1, 2, QW], BF16, tag="rec", name="rec")
                with nc.allow_low_precision(
                        reason="bf16 softmax denom reciprocal"):
                    nc.vector.reciprocal(out=rec, in_=ot[64:65, :, :])
                recb = psr.tile([64, 2, QW], F32, tag="recb", name="recb")
                nc.tensor.matmul(out=recb, lhsT=ones1, rhs=rec,
                                 start=True, stop=True)
                recb_sb = smallp.tile([64, 2, QW], F32, tag="recb_sb",
                                      name="recb_sb")
                nc.vector.tensor_copy(out=recb_sb, in_=recb)
                ysl = slice(512 * (n // 2), 512 * (n // 2 + 1))
                if h < 2:
                    ydst = YT01[64 * h:64 * (h + 1), ysl]
                else:
                    ydst = YT2[0:64, ysl]
                nc.vector.tensor_mul(ydst, ot[0:64, :, :], recb_sb)

            def proj_tile(t):
                """Output projection + fp16 z DMA for row tile t."""
                zt = zsp.tile([128, C], FP16, tag="zt", name="zt")
                for off, w in ((0, 512), (512, 256)):
                    acc = pss.tile([128, 512], F32, tag="s", name="accp")
                    nc.tensor.matmul(
                        out=acc[:, 0:w],
                        lhsT=YT01[:, 128 * t:128 * (t + 1)],
                        rhs=wp01[:, off:off + w],
                        start=True, stop=False)
                    nc.tensor.matmul(
                        out=acc[:, 0:w],
                        lhsT=YT2[0:64, 128 * t:128 * (t + 1)],
                        rhs=wp2[:, off:off + w],
                        start=False, stop=(not with_bias),
                        skip_group_check=True)
                    if with_bias:
                        nc.tensor.matmul(
                            out=acc[:, 0:w], lhsT=onesr[0:1, 0:128],
                            rhs=bp_s[0:1, off:off + w], start=False,
                            stop=True, skip_group_check=True)
                    if w == 512:
                        nc.scalar.activation(out=zt[:, 0:512],
                                             in_=acc[:, 0:512], func=AF.Copy)
                    else:
                        nc.vector.tensor_copy(out=zt[:, 512:768],
                                              in_=acc[:, 0:256])
                nc.sync.dma_start(out=z_out[128 * t:128 * (t + 1), :],
                                  in_=zt)

            # ---- fully pipelined emission ---------------------------------
            for m in range(4):
                qkv_block(m)
                for n in (2 * m, 2 * m + 1):
                    for h in range(HPC):
                        attn_chunk(h, n)
                    if n % 2 == 1:
                        for h in range(HPC):
                            normalize(h, n)
                        for t in range(4 * (n // 2), 4 * (n // 2 + 1)):
                            proj_tile(t)

    nc.finalize()
    return nc


def _prep_inputs(x, W_qkv, b_qkv, W_proj, b_proj):
    bf16 = ml_dtypes.bfloat16
    x = np.ascontiguousarray(np.asarray(x, dtype=np.float32))
    W_qkv = np.asarray(W_qkv, dtype=np.float32)
    b_qkv = np.asarray(b_qkv, dtype=np.float32)
    W_proj = np.asarray(W_proj, dtype=np.float32)
    b_proj = np.asarray(b_proj, dtype=np.float32)

    xb = [np.ascontiguousarray(x[b].astype(bf16)) for b in range(B)]
    ident = np.eye(128, dtype=np.float32).astype(bf16)

    p = np.arange(128)[:, None]
    q = np.arange(QW)[None, :]
    maskp = np.stack([(q >= p), (q >= p + 128)], axis=1)
    maskp = np.ascontiguousarray(maskp.astype(np.float32).astype(bf16))

    in_maps = []
    for d in range(NCORES):
        b, g = d // 4, d % 4
        ha, hb, hc = 3 * g, 3 * g + 1, 3 * g + 2
        # wq column order: [q_a q_b k_a k_b q_c k_c v_a v_b v_c]
        cols = []
        for blk, h in ((0, ha), (0, hb), (1, ha), (1, hb), (0, hc),
                       (1, hc), (2, ha), (2, hb), (2, hc)):
            base = blk * C + 64 * h
            cols.append(np.arange(base, base + 64))
        cols = np.concatenate(cols)
        wq_d = np.ascontiguousarray(W_qkv[:, cols].astype(bf16))
        rows = np.concatenate([np.arange(64 * h, 64 * h + 64)
                               for h in (ha, hb, hc)])
        wp_d = np.ascontiguousarray(W_proj[rows, :].astype(bf16))
        im = {
            "xbf": xb[b],
            "wq": wq_d,
            "wp": wp_d,
            "maskp": maskp,
            "ident_in": ident,
        }
        if np.any(b_qkv) or np.any(b_proj):
            im["bq"] = np.ascontiguousarray(
                b_qkv[cols].reshape(1, 576).astype(bf16))
            im["bp"] = np.ascontiguousarray(
                b_proj.reshape(1, C).astype(bf16))
        in_maps.append(im)
    return in_maps


def kernel(x, W_qkv, b_qkv, W_proj, b_proj):
    import os
    from concourse.bass_utils import run_bass_kernel_spmd

    in_maps = _prep_inputs(x, W_qkv, b_qkv, W_proj, b_proj)
    with_bias = bool(np.any(np.asarray(b_qkv)) or np.any(np.asarray(b_proj)))
    key = f"nc{with_bias}"
    if key not in _CACHE:
        _CACHE[key] = _build_program(with_bias)
    nc = _CACHE[key]
    res = run_bass_kernel_spmd(nc, in_maps, list(range(NCORES)),
                               trace=os.environ.get("KTRACE", "") == "1")
    _CACHE["last_result"] = res

    out = np.empty((B, T, C), dtype=np.float32)
    for b in range(B):
        acc = None
        for g in range(4):
            z = np.asarray(res.results[4 * b + g]["z"]).astype(np.float32)
            acc = z if acc is None else acc + z
        out[b] = acc
    return out
